# revision 1
# baseline (speedup 1.0000x reference)
"""Trainium2 Bass kernel for nn_FCNNShapeCounterValuationFunction.

Computes out[i] = 0.999 * a[i, int(z[i, 5])] for z:[B,32] f32, a:[B,16] f32.

Strategy (pure data parallel, 8 NeuronCores):
  - Shard rows across 8 cores (BC = B/8 rows each).
  - Per core, view rows as [128 partitions, BC/128] with per-partition
    contiguous blocks so every DMA descriptor is a large contiguous chunk
    (full HBM bandwidth; this problem is memory-bound: ~96 MB/core).
  - Per tile of F rows/partition: DMA z and a tiles in, extract the index
    column on ACT, then do the 16-way gather on DVE as 16 fused
    scalar_tensor_tensor ops  prod[:,k,:] = (idx == k) * a[:,:,k]
    followed by one strided tensor_reduce(add) over k. ACT applies the
    0.999 scale. All compute hides under the DMA stream.
"""

import numpy as np

B = 4194304
D = 32
K = 16
ATTR = 5
SCALE = 0.999
N_CORES = 8
P = 128
BC = B // N_CORES  # 524288 rows per core
F = 256  # rows per partition per tile

_cache = {}


def _build(bc=BC, f=F):
    """Build + compile the per-core Bass program for bc rows, tile size f."""
    from contextlib import ExitStack

    import concourse.bass as bass  # noqa: F401  (AP types come via handles)
    import concourse.tile as tile
    from concourse import bacc, mybir

    npp = bc // P  # rows per partition
    assert bc % P == 0 and npp % f == 0
    nt = npp // f

    nc = bacc.Bacc("TRN2", target_bir_lowering=False, debug=False, num_devices=N_CORES)
    z = nc.dram_tensor("z", [bc, D], mybir.dt.float32, kind="ExternalInput")
    a = nc.dram_tensor("a", [bc, K], mybir.dt.float32, kind="ExternalInput")
    out = nc.dram_tensor("out", [bc], mybir.dt.float32, kind="ExternalOutput")

    # Partition-major views: partition p owns rows [p*npp, (p+1)*npp) so each
    # partition's DMA chunk is contiguous in DRAM.
    zv = z.ap().rearrange("(p n) d -> p n d", p=P)
    av = a.ap().rearrange("(p n) k -> p n k", p=P)
    ov = out.ap().rearrange("(p n) -> p n", p=P)

    f32 = mybir.dt.float32
    eq = mybir.AluOpType.is_equal
    mult = mybir.AluOpType.mult
    add = mybir.AluOpType.add

    with ExitStack() as ctx:
        tc = ctx.enter_context(tile.TileContext(nc))
        zpool = ctx.enter_context(tc.tile_pool(name="zpool", bufs=3))
        apool = ctx.enter_context(tc.tile_pool(name="apool", bufs=3))
        ppool = ctx.enter_context(tc.tile_pool(name="ppool", bufs=1))
        spool = ctx.enter_context(tc.tile_pool(name="spool", bufs=3))

        for t in range(nt):
            lo, hi = t * f, (t + 1) * f
            zt = zpool.tile([P, f, D], f32)
            nc.sync.dma_start(zt[:], zv[:, lo:hi, :])
            at = apool.tile([P, f, K], f32)
            nc.sync.dma_start(at[:], av[:, lo:hi, :])

            # Extract the (float-encoded) index column; frees zt for reuse.
            idx = spool.tile([P, f], f32)
            nc.scalar.copy(idx[:], zt[:, :, ATTR])

            # prod[:, k, :] = (idx == k) * a[:, :, k]   (k-major so the stt
            # output is contiguous)
            prod = ppool.tile([P, K, f], f32)
            for k in range(K):
                nc.vector.scalar_tensor_tensor(
                    prod[:, k, :], idx[:], float(k), at[:, :, k], eq, mult
                )

            # out[p, n] = sum_k prod[p, k, n]  (strided view puts k innermost)
            red = spool.tile([P, f], f32)
            nc.vector.tensor_reduce(
                red[:], prod.rearrange("p k n -> p n k"), axis=mybir.AxisListType.X, op=add
            )

            sc = spool.tile([P, f], f32)
            nc.scalar.mul(sc[:], red[:], SCALE)
            nc.scalar.dma_start(ov[:, lo:hi], sc[:])

    nc.compile()
    return nc


def _get(bc=BC, f=F):
    key = (bc, f)
    if key not in _cache:
        _cache[key] = _build(bc, f)
    return _cache[key]


def kernel(z, a, attr_index=5, **run_kwargs):
    """Full inputs in, full output out. Shards rows over 8 NeuronCores."""
    from concourse import bass_utils

    assert int(attr_index) == ATTR
    z = np.asarray(z, dtype=np.float32)
    a = np.asarray(a, dtype=np.float32)
    assert z.shape == (B, D) and a.shape == (B, K)

    nc = _get()
    in_maps = [
        {"z": z[c * BC : (c + 1) * BC], "a": a[c * BC : (c + 1) * BC]}
        for c in range(N_CORES)
    ]
    res = bass_utils.run_bass_kernel_spmd(
        nc, in_maps, core_ids=list(range(N_CORES)), **run_kwargs
    )
    out = np.concatenate([r["out"] for r in res.results], axis=0)
    if run_kwargs:
        kernel.last_results = res
    return out


# revision 3
# speedup vs baseline: 1.0929x; 1.0929x over previous
"""Trainium2 Bass kernel for nn_FCNNShapeCounterValuationFunction.

Computes out[i] = 0.999 * a[i, int(z[i, 5])] for z:[B,32] f32, a:[B,16] f32.

Strategy (pure data parallel, 8 NeuronCores):
  - Shard rows across 8 cores (BC = B/8 rows each).
  - Per core, view rows as [128 partitions, BC/128] with per-partition
    contiguous blocks so every DMA descriptor is a large contiguous chunk
    (full HBM bandwidth; this problem is memory-bound: ~96 MB/core).
  - Per tile of F rows/partition: DMA z and a tiles in, extract the index
    column on ACT, then do the 16-way gather on DVE as 16 fused
    scalar_tensor_tensor ops  prod[:,k,:] = (idx == k) * a[:,:,k]
    followed by one strided tensor_reduce(add) over k. ACT applies the
    0.999 scale. All compute hides under the DMA stream.
"""

import numpy as np

B = 4194304
D = 32
K = 16
ATTR = 5
SCALE = 0.999
N_CORES = 8
P = 128
BC = B // N_CORES  # 524288 rows per core
F = 256  # rows per partition per tile

_cache = {}


def _round_sizes(npp):
    """Compute rounds (rows/partition each). Mostly 512-row rounds to
    amortize DVE per-op overhead; the final 512 is split 256/128/128 to
    shorten the post-DMA compute tail."""
    assert npp % 512 == 0 and npp >= 512
    if npp == 512:
        return [256, 128, 128]
    return [512] * (npp // 512 - 1) + [256, 128, 128]


def _build(bc=BC, f=F):
    """Build + compile the per-core Bass program for bc rows."""
    from contextlib import ExitStack

    import concourse.tile as tile
    from concourse import bacc, mybir

    npp = bc // P  # rows per partition
    assert bc % P == 0
    rounds = _round_sizes(npp)

    nc = bacc.Bacc("TRN2", target_bir_lowering=False, debug=False, num_devices=N_CORES)
    z = nc.dram_tensor("z", [bc, D], mybir.dt.float32, kind="ExternalInput")
    a = nc.dram_tensor("a", [bc, K], mybir.dt.float32, kind="ExternalInput")
    out = nc.dram_tensor("out", [bc], mybir.dt.float32, kind="ExternalOutput")

    # Partition-major views: partition p owns rows [p*npp, (p+1)*npp) so each
    # partition's DMA chunk is contiguous in DRAM.
    zv = z.ap().rearrange("(p n) d -> p n d", p=P)
    av = a.ap().rearrange("(p n) k -> p n k", p=P)
    ov = out.ap().rearrange("(p n) -> p n", p=P)

    f32 = mybir.dt.float32
    eq = mybir.AluOpType.is_equal
    mult = mybir.AluOpType.mult
    add = mybir.AluOpType.add
    FZ = 256  # rows/partition per z DMA tile (32 KB/partition chunks)

    with ExitStack() as ctx:
        tc = ctx.enter_context(tile.TileContext(nc))
        zpool = ctx.enter_context(tc.tile_pool(name="zpool", bufs=2))
        apool = ctx.enter_context(tc.tile_pool(name="apool", bufs=2))
        ppool = ctx.enter_context(tc.tile_pool(name="ppool", bufs=1))
        spool = ctx.enter_context(tc.tile_pool(name="spool", bufs=2))

        pos = 0
        for f in rounds:
            lo, hi = pos, pos + f
            pos = hi

            # z arrives in <=FZ-row tiles; idx collects the index column.
            idx = spool.tile([P, f], f32, tag="idx")
            for zlo in range(lo, hi, FZ):
                zhi = min(zlo + FZ, hi)
                zt = zpool.tile([P, zhi - zlo, D], f32, tag="zt")
                nc.sync.dma_start(zt[:], zv[:, zlo:zhi, :])
                nc.scalar.copy(idx[:, zlo - lo : zhi - lo], zt[:, :, ATTR])

            at = apool.tile([P, f, K], f32, tag="at")
            nc.sync.dma_start(at[:], av[:, lo:hi, :])

            # prod[:, k, :] = (idx == k) * a[:, :, k]   (k-major: contiguous out)
            prod = ppool.tile([P, K, f], f32, tag="prod")
            for k in range(K):
                nc.vector.scalar_tensor_tensor(
                    prod[:, k, :], idx[:], float(k), at[:, :, k], eq, mult
                )

            # In-place binary-tree sum over k: all operands contiguous.
            for h in (8, 4, 2):
                nc.vector.tensor_tensor(
                    prod[:, :h, :], prod[:, :h, :], prod[:, h : 2 * h, :], add
                )
            red = spool.tile([P, f], f32, tag="red")
            nc.vector.tensor_tensor(red[:], prod[:, 0, :], prod[:, 1, :], add)

            sc = spool.tile([P, f], f32, tag="sc")
            nc.scalar.mul(sc[:], red[:], SCALE)
            nc.scalar.dma_start(ov[:, lo:hi], sc[:])

    nc.compile()
    return nc


def _get(bc=BC, f=F):
    key = (bc, f)
    if key not in _cache:
        _cache[key] = _build(bc, f)
    return _cache[key]


def kernel(z, a, attr_index=5, **run_kwargs):
    """Full inputs in, full output out. Shards rows over 8 NeuronCores."""
    from concourse import bass_utils

    assert int(attr_index) == ATTR
    z = np.asarray(z, dtype=np.float32)
    a = np.asarray(a, dtype=np.float32)
    assert z.shape == (B, D) and a.shape == (B, K)

    nc = _get()
    in_maps = [
        {"z": z[c * BC : (c + 1) * BC], "a": a[c * BC : (c + 1) * BC]}
        for c in range(N_CORES)
    ]
    res = bass_utils.run_bass_kernel_spmd(
        nc, in_maps, core_ids=list(range(N_CORES)), **run_kwargs
    )
    out = np.concatenate([r["out"] for r in res.results], axis=0)
    if run_kwargs:
        kernel.last_results = res
    return out
